# revision 2
# baseline (speedup 1.0000x reference)
"""FFM layer kernel for Trainium2, data-parallel over batch on 8 NeuronCores.

The reference computes, for each sample b:
    x = [dense(13) | onehot(26 fields x 1000)]            # [B, 26013]
    linear = w0 + x @ w                                   # [B, 1]
    field_f = einsum('bf,fik->bik', x, v)                 # [B, 39, 8]
    inter = 0.5*((sum_i field_f)^2.sum(k) - (field_f^2).sum(i,k))
    out = linear + inter

Because x is one-hot in the sparse block, x @ [v|w] is a 26-row gather from
the [26013, 313] table (312 = 39*8 flattened v row, col 312 = w) plus a tiny
dense [14]x[14,313] matmul (row 13 = ones row carrying w0 into col 312).
Each core handles 512 samples; the gather is one indirect DMA per 128-sample
tile fetching 26 rows of 1252 B per partition.
"""

import numpy as np

N_DENSE = 13
N_SPARSE = 26
ONEHOT = 1000
FIELD = 39
K = 8
FEAT = N_DENSE + N_SPARSE * ONEHOT  # 26013
B = 4096
NCORES = 8
BC = B // NCORES  # 512 samples per core
P = 128
NT = BC // P  # 4 tiles per core
D = FIELD * K  # 312
DW = D + 1  # 313 (col 312 carries the linear weight)
SQRT_HALF = 0.7071067811865476

_cached_nc = None


def _build_program():
    global _cached_nc
    if _cached_nc is not None:
        return _cached_nc

    import concourse.bass as bass
    import concourse.bacc as bacc
    import concourse.mybir as mybir
    from concourse.tile import TileContext

    nc = bacc.Bacc(
        "TRN2",
        debug=False,
        enable_asserts=False,
        target_bir_lowering=False,
        num_devices=NCORES,
    )
    f32 = mybir.dt.float32
    table = nc.dram_tensor("table", [FEAT, DW], f32, kind="ExternalInput").ap()
    idx = nc.dram_tensor("idx", [P, NT * N_SPARSE], mybir.dt.int32, kind="ExternalInput").ap()
    dnt = nc.dram_tensor("dnt", [N_DENSE + 1, BC], f32, kind="ExternalInput").ap()
    vdx = nc.dram_tensor("vdx", [N_DENSE + 1, DW], f32, kind="ExternalInput").ap()
    out = nc.dram_tensor("out", [BC, 1], f32, kind="ExternalOutput").ap()

    with TileContext(nc) as tc:
        with tc.tile_pool(name="const", bufs=1) as cpool, \
             tc.tile_pool(name="gath", bufs=3) as gpool, \
             tc.tile_pool(name="work", bufs=2) as wpool, \
             tc.tile_pool(name="psum", bufs=2, space="PSUM") as ppool:
            idx_sb = cpool.tile([P, NT * N_SPARSE], mybir.dt.int32)
            nc.sync.dma_start(out=idx_sb[:], in_=idx[:])
            dnt_sb = cpool.tile([N_DENSE + 1, BC], f32)
            nc.sync.dma_start(out=dnt_sb[:], in_=dnt[:])
            vdx_sb = cpool.tile([N_DENSE + 1, DW], f32)
            nc.sync.dma_start(out=vdx_sb[:], in_=vdx[:])

            for t in range(NT):
                # 26 embedding rows per sample; HW indirect DMA consumes one
                # offset per partition, so gather one field per DMA:
                # g[p, c*313:(c+1)*313] = table[idx[p, t*26+c], :]
                g = gpool.tile([P, N_SPARSE * DW], f32, tag="g")
                for c in range(N_SPARSE):
                    nc.gpsimd.indirect_dma_start(
                        out=g[:, c * DW:(c + 1) * DW],
                        out_offset=None,
                        in_=table[:],
                        in_offset=bass.IndirectOffsetOnAxis(
                            ap=idx_sb[:, t * N_SPARSE + c:t * N_SPARSE + c + 1],
                            axis=0,
                        ),
                    )
                # dense + w0 contribution: [128, 313]
                ps = ppool.tile([P, DW], f32, tag="ps")
                nc.tensor.matmul(
                    out=ps[:],
                    lhsT=dnt_sb[:, t * P:(t + 1) * P],
                    rhs=vdx_sb[:],
                    start=True,
                    stop=True,
                )
                # sum the 26 gathered rows: view [P, 313, 26], reduce innermost
                ssum = wpool.tile([P, DW], f32, tag="ssum")
                gv = g[:].rearrange("p (c d) -> p d c", c=N_SPARSE)
                nc.vector.reduce_sum(out=ssum[:], in_=gv, axis=mybir.AxisListType.X)
                tot = wpool.tile([P, DW], f32, tag="tot")
                nc.vector.tensor_tensor(
                    out=tot[:], in0=ssum[:], in1=ps[:], op=mybir.AluOpType.add
                )
                # s_k = sum_i field_f[i, k]: view [P, 8, 39], reduce innermost
                s8 = wpool.tile([P, K], f32, tag="s8")
                tv = tot[:, :D].rearrange("p (i k) -> p k i", k=K)
                nc.vector.reduce_sum(out=s8[:], in_=tv, axis=mybir.AxisListType.X)
                # 0.5 * sum of squares, fused on the scalar engine
                sq = wpool.tile([P, D], f32, tag="sq")
                h_sumsq = wpool.tile([P, 1], f32, tag="h_sumsq")
                nc.scalar.activation(
                    out=sq[:], in_=tot[:, :D],
                    func=mybir.ActivationFunctionType.Square,
                    scale=SQRT_HALF, accum_out=h_sumsq[:],
                )
                sq8 = wpool.tile([P, K], f32, tag="sq8")
                h_ssq = wpool.tile([P, 1], f32, tag="h_ssq")
                nc.scalar.activation(
                    out=sq8[:], in_=s8[:],
                    func=mybir.ActivationFunctionType.Square,
                    scale=SQRT_HALF, accum_out=h_ssq[:],
                )
                ot = wpool.tile([P, 1], f32, tag="ot")
                nc.vector.tensor_tensor(
                    out=ot[:], in0=h_ssq[:], in1=h_sumsq[:],
                    op=mybir.AluOpType.subtract,
                )
                nc.vector.tensor_tensor(
                    out=ot[:], in0=ot[:], in1=tot[:, D:DW],
                    op=mybir.AluOpType.add,
                )
                nc.sync.dma_start(out=out[t * P:(t + 1) * P, :], in_=ot[:])

    nc.compile()
    _cached_nc = nc
    return nc


def _prepare_inputs(inputs, w0, w, v):
    dense = np.ascontiguousarray(inputs[:, :N_DENSE].astype(np.float32))
    idx = inputs[:, N_DENSE:].astype(np.int32)
    flat_idx = (N_DENSE + np.arange(N_SPARSE, dtype=np.int32) * ONEHOT)[None, :] + idx

    table = np.concatenate(
        [v.reshape(FEAT, D).astype(np.float32), w.reshape(FEAT, 1).astype(np.float32)],
        axis=1,
    )
    table = np.ascontiguousarray(table)
    w0_row = np.zeros((1, DW), np.float32)
    w0_row[0, D] = np.asarray(w0, np.float32).reshape(-1)[0]
    vdx = np.ascontiguousarray(np.concatenate([table[:N_DENSE], w0_row], axis=0))

    in_maps = []
    for c in range(NCORES):
        sl = slice(c * BC, (c + 1) * BC)
        dnt = np.concatenate(
            [dense[sl].T, np.ones((1, BC), np.float32)], axis=0
        )  # [14, 512]
        fi = (
            flat_idx[sl]
            .reshape(NT, P, N_SPARSE)
            .transpose(1, 0, 2)
            .reshape(P, NT * N_SPARSE)
        )
        in_maps.append(
            {
                "table": table,
                "idx": np.ascontiguousarray(fi),
                "dnt": np.ascontiguousarray(dnt),
                "vdx": vdx,
            }
        )
    return in_maps


def kernel(**inputs):
    from concourse import bass_utils

    nc = _build_program()
    in_maps = _prepare_inputs(
        np.asarray(inputs["inputs"]),
        np.asarray(inputs["w0"]),
        np.asarray(inputs["w"]),
        np.asarray(inputs["v"]),
    )
    res = bass_utils.run_bass_kernel_spmd(nc, in_maps, core_ids=list(range(NCORES)))
    outs = [np.asarray(res.results[c]["out"]) for c in range(NCORES)]
    return np.concatenate(outs, axis=0).astype(np.float32)


# revision 4
# speedup vs baseline: 1.1416x; 1.1416x over previous
"""FFM layer kernel for Trainium2, data-parallel over batch on 8 NeuronCores.

The reference computes, for each sample b:
    x = [dense(13) | onehot(26 fields x 1000)]            # [B, 26013]
    linear = w0 + x @ w                                   # [B, 1]
    field_f = einsum('bf,fik->bik', x, v)                 # [B, 39, 8]
    inter = 0.5*((sum_i field_f)^2.sum(k) - (field_f^2).sum(i,k))
    out = linear + inter

Because x is one-hot in the sparse block, x @ [v|w] is a 26-row gather from
a [26013, 320] table (cols 0..311 = flattened v row, col 312 = w, 313.. pad)
plus a tiny dense [14]x[14,313] matmul (row 13 = ones row carrying w0 into
col 312).  Each core handles 512 samples as 4 tiles of 128; each tile's 26
rows/sample are fetched by one gpsimd dma_gather (3328 rows of 1280 B).
"""

import numpy as np

N_DENSE = 13
N_SPARSE = 26
ONEHOT = 1000
FIELD = 39
K = 8
FEAT = N_DENSE + N_SPARSE * ONEHOT  # 26013
B = 4096
NCORES = 8
BC = B // NCORES  # 512 samples per core
P = 128
NT = BC // P  # 4 tiles per core
D = FIELD * K  # 312
DW = D + 1  # 313 (col 312 carries the linear weight)
E = 320  # gathered row width, padded so the 1280 B row is a multiple of 256
NI = N_SPARSE * P  # 3328 indices per tile gather
SQRT_HALF = 0.7071067811865476

_cached_nc = None


def _build_program():
    global _cached_nc
    if _cached_nc is not None:
        return _cached_nc

    import concourse.bacc as bacc
    import concourse.mybir as mybir
    from concourse.tile import TileContext
    from concourse import library_config

    nc = bacc.Bacc(
        "TRN2",
        debug=False,
        enable_asserts=False,
        target_bir_lowering=False,
        num_devices=NCORES,
    )
    f32 = mybir.dt.float32
    i16 = mybir.dt.int16
    table = nc.dram_tensor("table", [FEAT, E], f32, kind="ExternalInput").ap()
    idx = nc.dram_tensor("idx", [P, NT * NI // 16], i16, kind="ExternalInput").ap()
    dnt = nc.dram_tensor("dnt", [N_DENSE + 1, BC], f32, kind="ExternalInput").ap()
    vdx = nc.dram_tensor("vdx", [N_DENSE + 1, DW], f32, kind="ExternalInput").ap()
    out = nc.dram_tensor("out", [BC, 1], f32, kind="ExternalOutput").ap()

    with TileContext(nc) as tc:
        with tc.tile_pool(name="const", bufs=1) as cpool, \
             tc.tile_pool(name="gath", bufs=2) as gpool, \
             tc.tile_pool(name="work", bufs=2) as wpool, \
             tc.tile_pool(name="psum", bufs=2, space="PSUM") as ppool:
            nc.gpsimd.load_library(library_config.mlp)
            idx_sb = cpool.tile([P, NT * NI // 16], i16)
            nc.sync.dma_start(out=idx_sb[:], in_=idx[:])
            dnt_sb = cpool.tile([N_DENSE + 1, BC], f32)
            nc.sync.dma_start(out=dnt_sb[:], in_=dnt[:])
            vdx_sb = cpool.tile([N_DENSE + 1, DW], f32)
            nc.sync.dma_start(out=vdx_sb[:], in_=vdx[:])

            for t in range(NT):
                # g[p, c, :] = table[flat_idx[t*128 + p, c], :]
                g = gpool.tile([P, N_SPARSE * E], f32, tag="g")
                g3 = g[:].rearrange("p (c e) -> p c e", e=E)
                nc.gpsimd.dma_gather(
                    g3,
                    table[:],
                    idx_sb[:, t * (NI // 16):(t + 1) * (NI // 16)],
                    NI,
                    NI,
                    E,
                    single_packet=False,
                )
                # dense + w0 contribution: [128, 313]
                ps = ppool.tile([P, DW], f32, tag="ps")
                nc.tensor.matmul(
                    out=ps[:],
                    lhsT=dnt_sb[:, t * P:(t + 1) * P],
                    rhs=vdx_sb[:],
                    start=True,
                    stop=True,
                )
                # sum the 26 gathered rows with a contiguous pairwise tree
                # (fp32 tensor_tensor runs 1 elem/cycle; contiguous > strided)
                add = lambda o, a, b: nc.vector.tensor_tensor(
                    out=o, in0=a, in1=b, op=mybir.AluOpType.add
                )
                a13 = wpool.tile([P, 13 * E], f32, tag="a13")
                add(a13[:], g[:, :13 * E], g[:, 13 * E:26 * E])
                b6 = wpool.tile([P, 6 * E], f32, tag="b6")
                add(b6[:], a13[:, :6 * E], a13[:, 6 * E:12 * E])
                c3 = wpool.tile([P, 3 * E], f32, tag="c3")
                add(c3[:], b6[:, :3 * E], b6[:, 3 * E:6 * E])
                d1 = wpool.tile([P, E], f32, tag="d1")
                add(d1[:], c3[:, :E], c3[:, E:2 * E])
                add(d1[:], d1[:], c3[:, 2 * E:3 * E])
                add(d1[:], d1[:], a13[:, 12 * E:13 * E])
                tot = wpool.tile([P, DW], f32, tag="tot")
                add(tot[:], d1[:, :DW], ps[:])
                # s_k = sum_i field_f[i, k]: view [P, 8, 39], reduce innermost
                s8 = wpool.tile([P, K], f32, tag="s8")
                tv = tot[:, :D].rearrange("p (i k) -> p k i", k=K)
                nc.vector.reduce_sum(out=s8[:], in_=tv, axis=mybir.AxisListType.X)
                # 0.5 * sum of squares, fused on the scalar engine
                sq = wpool.tile([P, D], f32, tag="sq")
                h_sumsq = wpool.tile([P, 1], f32, tag="h_sumsq")
                nc.scalar.activation(
                    out=sq[:], in_=tot[:, :D],
                    func=mybir.ActivationFunctionType.Square,
                    scale=SQRT_HALF, accum_out=h_sumsq[:],
                )
                sq8 = wpool.tile([P, K], f32, tag="sq8")
                h_ssq = wpool.tile([P, 1], f32, tag="h_ssq")
                nc.scalar.activation(
                    out=sq8[:], in_=s8[:],
                    func=mybir.ActivationFunctionType.Square,
                    scale=SQRT_HALF, accum_out=h_ssq[:],
                )
                ot = wpool.tile([P, 1], f32, tag="ot")
                nc.vector.tensor_tensor(
                    out=ot[:], in0=h_ssq[:], in1=h_sumsq[:],
                    op=mybir.AluOpType.subtract,
                )
                add(ot[:], ot[:], tot[:, D:DW])
                nc.sync.dma_start(out=out[t * P:(t + 1) * P, :], in_=ot[:])

    nc.compile()
    _cached_nc = nc
    return nc


def _prepare_inputs(inputs, w0, w, v):
    dense = np.ascontiguousarray(inputs[:, :N_DENSE].astype(np.float32))
    idx = inputs[:, N_DENSE:].astype(np.int32)
    flat_idx = (N_DENSE + np.arange(N_SPARSE, dtype=np.int32) * ONEHOT)[None, :] + idx

    table = np.zeros((FEAT, E), np.float32)
    table[:, :D] = v.reshape(FEAT, D)
    table[:, D] = np.asarray(w, np.float32).reshape(FEAT)
    w0_row = np.zeros((1, DW), np.float32)
    w0_row[0, D] = np.asarray(w0, np.float32).reshape(-1)[0]
    vdx = np.ascontiguousarray(np.concatenate([table[:N_DENSE, :DW], w0_row], axis=0))

    in_maps = []
    for c in range(NCORES):
        sl = slice(c * BC, (c + 1) * BC)
        dnt = np.concatenate(
            [dense[sl].T, np.ones((1, BC), np.float32)], axis=0
        )  # [14, 512]
        # per tile t the gather consumes indices i = c*128 + p, laid out
        # int16 at [i % 16, i // 16] in the first 16 partitions, replicated
        # 8x down the partitions (one copy per Q7 core)
        fi = flat_idx[sl].astype(np.int16)  # [512, 26]
        blocks = []
        for t in range(NT):
            lin = fi[t * P:(t + 1) * P].T.reshape(NI)  # i = c*128 + p
            blk = lin.reshape(NI // 16, 16).T  # [16, NI/16]
            blocks.append(np.tile(blk, (8, 1)))  # [128, NI/16]
        idx_buf = np.ascontiguousarray(np.concatenate(blocks, axis=1))
        in_maps.append(
            {
                "table": table,
                "idx": idx_buf,
                "dnt": np.ascontiguousarray(dnt),
                "vdx": vdx,
            }
        )
    return in_maps


def kernel(**inputs):
    from concourse import bass_utils

    nc = _build_program()
    in_maps = _prepare_inputs(
        np.asarray(inputs["inputs"]),
        np.asarray(inputs["w0"]),
        np.asarray(inputs["w"]),
        np.asarray(inputs["v"]),
    )
    res = bass_utils.run_bass_kernel_spmd(nc, in_maps, core_ids=list(range(NCORES)))
    outs = [np.asarray(res.results[c]["out"]) for c in range(NCORES)]
    return np.concatenate(outs, axis=0).astype(np.float32)


# revision 7
# speedup vs baseline: 1.2744x; 1.1164x over previous
"""FFM layer kernel for Trainium2, data-parallel over batch on 8 NeuronCores.

The reference computes, for each sample b:
    x = [dense(13) | onehot(26 fields x 1000)]            # [B, 26013]
    linear = w0 + x @ w                                   # [B, 1]
    field_f = einsum('bf,fik->bik', x, v)                 # [B, 39, 8]
    inter = 0.5*((sum_i field_f)^2.sum(k) - (field_f^2).sum(i,k))
    out = linear + inter

Because x is one-hot in the sparse block, x @ [v|w] is a 26-row gather from
a [26013, 320] table (cols 0..311 = flattened v row, col 312 = w, 313.. pad)
plus a tiny dense [14]x[14,313] matmul (row 13 = ones row carrying w0 into
col 312).  Each core handles 512 samples as 4 tiles of 128; each tile's 26
rows/sample are fetched by one gpsimd dma_gather (3328 rows of 1280 B).
"""

import numpy as np

N_DENSE = 13
N_SPARSE = 26
ONEHOT = 1000
FIELD = 39
K = 8
FEAT = N_DENSE + N_SPARSE * ONEHOT  # 26013
B = 4096
NCORES = 8
BC = B // NCORES  # 512 samples per core
P = 128
NT = BC // P  # 4 tiles per core
D = FIELD * K  # 312
DW = D + 1  # 313 (col 312 carries the linear weight)
E = 320  # gathered row width, padded so the 1280 B row is a multiple of 256
NI = N_SPARSE * P  # 3328 indices per tile gather
SQRT_HALF = 0.7071067811865476

_cached_nc = None


def _build_program():
    global _cached_nc
    if _cached_nc is not None:
        return _cached_nc

    import concourse.bacc as bacc
    import concourse.mybir as mybir
    from concourse.tile import TileContext
    from concourse import library_config

    nc = bacc.Bacc(
        "TRN2",
        debug=False,
        enable_asserts=False,
        target_bir_lowering=False,
        num_devices=NCORES,
        num_swdge_queues=4,
    )
    f32 = mybir.dt.float32
    i16 = mybir.dt.int16
    table = nc.dram_tensor("table", [FEAT, E], f32, kind="ExternalInput").ap()
    idx = nc.dram_tensor("idx", [P, NT * NI // 16], i16, kind="ExternalInput").ap()
    dnt = nc.dram_tensor("dnt", [N_DENSE + 1, BC], f32, kind="ExternalInput").ap()
    vdx = nc.dram_tensor("vdx", [N_DENSE + 1, DW], f32, kind="ExternalInput").ap()
    out = nc.dram_tensor("out", [BC, 1], f32, kind="ExternalOutput").ap()

    with TileContext(nc) as tc:
        with tc.tile_pool(name="const", bufs=1) as cpool, \
             tc.tile_pool(name="gath", bufs=2) as gpool, \
             tc.tile_pool(name="work", bufs=2) as wpool, \
             tc.tile_pool(name="psum", bufs=2, space="PSUM") as ppool:
            nc.gpsimd.load_library(library_config.mlp)
            idx_sb = cpool.tile([P, NT * NI // 16], i16)
            nc.sync.dma_start(out=idx_sb[:], in_=idx[:])
            dnt_sb = cpool.tile([N_DENSE + 1, BC], f32)
            nc.sync.dma_start(out=dnt_sb[:], in_=dnt[:])
            vdx_sb = cpool.tile([N_DENSE + 1, DW], f32)
            nc.sync.dma_start(out=vdx_sb[:], in_=vdx[:])

            for t in range(NT):
                # gh[h][p, c, :] = table[flat_idx[t*128 + p, 13*h + c], :]
                # two half-gathers per tile on rotating SWDGE queues so the
                # adder tree can start on half 0 while half 1 streams in
                HNI = NI // 2  # 1664 indices per half
                HC = HNI // 16  # idx columns per half
                gh = []
                for h in range(2):
                    g = gpool.tile([P, 13 * E], f32, tag=f"g{h}")
                    g3 = g[:].rearrange("p (c e) -> p c e", e=E)
                    col = (2 * t + h) * HC
                    nc.gpsimd.dma_gather(
                        g3,
                        table[:],
                        idx_sb[:, col:col + HC],
                        HNI,
                        HNI,
                        E,
                        single_packet=False,
                        queue_num=(2 * t + h) % 4,
                    )
                    gh.append(g)
                # dense + w0 contribution: [128, 313]
                ps = ppool.tile([P, DW], f32, tag="ps")
                nc.tensor.matmul(
                    out=ps[:],
                    lhsT=dnt_sb[:, t * P:(t + 1) * P],
                    rhs=vdx_sb[:],
                    start=True,
                    stop=True,
                )
                # sum the 26 gathered rows with a contiguous pairwise tree
                # (fp32 tensor_tensor runs 1 elem/cycle; contiguous > strided)
                add = lambda o, a, b: nc.vector.tensor_tensor(
                    out=o, in0=a, in1=b, op=mybir.AluOpType.add
                )
                b3 = []
                for h in range(2):
                    a6 = wpool.tile([P, 6 * E], f32, tag=f"a6_{h}")
                    add(a6[:], gh[h][:, :6 * E], gh[h][:, 6 * E:12 * E])
                    b = wpool.tile([P, 3 * E], f32, tag=f"b3_{h}")
                    add(b[:], a6[:, :3 * E], a6[:, 3 * E:6 * E])
                    b3.append(b)
                c3 = wpool.tile([P, 3 * E], f32, tag="c3")
                add(c3[:], b3[0][:], b3[1][:])
                d1 = wpool.tile([P, E], f32, tag="d1")
                add(d1[:], c3[:, :E], c3[:, E:2 * E])
                add(d1[:], d1[:], c3[:, 2 * E:3 * E])
                add(d1[:], d1[:], gh[0][:, 12 * E:13 * E])
                add(d1[:], d1[:], gh[1][:, 12 * E:13 * E])
                tot = wpool.tile([P, DW], f32, tag="tot")
                add(tot[:], d1[:, :DW], ps[:])
                # s_k = sum_i field_f[i, k]: view [P, 8, 39], reduce innermost
                s8 = wpool.tile([P, K], f32, tag="s8")
                tv = tot[:, :D].rearrange("p (i k) -> p k i", k=K)
                nc.vector.reduce_sum(out=s8[:], in_=tv, axis=mybir.AxisListType.X)
                # 0.5 * sum of squares, fused on the scalar engine
                sq = wpool.tile([P, D], f32, tag="sq")
                h_sumsq = wpool.tile([P, 1], f32, tag="h_sumsq")
                nc.scalar.activation(
                    out=sq[:], in_=tot[:, :D],
                    func=mybir.ActivationFunctionType.Square,
                    scale=SQRT_HALF, accum_out=h_sumsq[:],
                )
                sq8 = wpool.tile([P, K], f32, tag="sq8")
                h_ssq = wpool.tile([P, 1], f32, tag="h_ssq")
                nc.scalar.activation(
                    out=sq8[:], in_=s8[:],
                    func=mybir.ActivationFunctionType.Square,
                    scale=SQRT_HALF, accum_out=h_ssq[:],
                )
                ot = wpool.tile([P, 1], f32, tag="ot")
                nc.vector.tensor_tensor(
                    out=ot[:], in0=h_ssq[:], in1=h_sumsq[:],
                    op=mybir.AluOpType.subtract,
                )
                add(ot[:], ot[:], tot[:, D:DW])
                nc.sync.dma_start(out=out[t * P:(t + 1) * P, :], in_=ot[:])

    nc.compile()
    _cached_nc = nc
    return nc


def _prepare_inputs(inputs, w0, w, v):
    dense = np.ascontiguousarray(inputs[:, :N_DENSE].astype(np.float32))
    idx = inputs[:, N_DENSE:].astype(np.int32)
    flat_idx = (N_DENSE + np.arange(N_SPARSE, dtype=np.int32) * ONEHOT)[None, :] + idx

    table = np.zeros((FEAT, E), np.float32)
    table[:, :D] = v.reshape(FEAT, D)
    table[:, D] = np.asarray(w, np.float32).reshape(FEAT)
    w0_row = np.zeros((1, DW), np.float32)
    w0_row[0, D] = np.asarray(w0, np.float32).reshape(-1)[0]
    vdx = np.ascontiguousarray(np.concatenate([table[:N_DENSE, :DW], w0_row], axis=0))

    in_maps = []
    for c in range(NCORES):
        sl = slice(c * BC, (c + 1) * BC)
        dnt = np.concatenate(
            [dense[sl].T, np.ones((1, BC), np.float32)], axis=0
        )  # [14, 512]
        # per tile t the gather consumes indices i = c*128 + p, laid out
        # int16 at [i % 16, i // 16] in the first 16 partitions, replicated
        # 8x down the partitions (one copy per Q7 core)
        fi = flat_idx[sl].astype(np.int16)  # [512, 26]
        blocks = []
        for t in range(NT):
            for h in range(2):
                # half h covers fields 13h..13h+12; order i = c_local*128 + p
                lin = fi[t * P:(t + 1) * P, 13 * h:13 * (h + 1)].T.reshape(NI // 2)
                blk = lin.reshape(NI // 32, 16).T  # [16, HNI/16]
                blocks.append(np.tile(blk, (8, 1)))  # [128, HNI/16]
        idx_buf = np.ascontiguousarray(np.concatenate(blocks, axis=1))
        in_maps.append(
            {
                "table": table,
                "idx": idx_buf,
                "dnt": np.ascontiguousarray(dnt),
                "vdx": vdx,
            }
        )
    return in_maps


def kernel(**inputs):
    from concourse import bass_utils

    nc = _build_program()
    in_maps = _prepare_inputs(
        np.asarray(inputs["inputs"]),
        np.asarray(inputs["w0"]),
        np.asarray(inputs["w"]),
        np.asarray(inputs["v"]),
    )
    res = bass_utils.run_bass_kernel_spmd(nc, in_maps, core_ids=list(range(NCORES)))
    outs = [np.asarray(res.results[c]["out"]) for c in range(NCORES)]
    return np.concatenate(outs, axis=0).astype(np.float32)


# revision 9
# speedup vs baseline: 1.6537x; 1.2976x over previous
"""FFM layer kernel for Trainium2, data-parallel over batch on 8 NeuronCores.

The reference computes, for each sample b:
    x = [dense(13) | onehot(26 fields x 1000)]            # [B, 26013]
    linear = w0 + x @ w                                   # [B, 1]
    field_f = einsum('bf,fik->bik', x, v)                 # [B, 39, 8]
    inter = 0.5*((sum_i field_f)^2.sum(k) - (field_f^2).sum(i,k))
    out = linear + inter

Because x is one-hot in the sparse block, x @ [v|w] is a 26-row gather from
a [26013, 320] table (cols 0..311 = flattened v row, col 312 = w, 313.. pad)
plus a tiny dense [14]x[14,313] matmul (row 13 = ones row carrying w0 into
col 312).  Each core handles 512 samples as 4 tiles of 128; each tile's 26
rows/sample are fetched by one gpsimd dma_gather (3328 rows of 1280 B).
"""

import numpy as np

N_DENSE = 13
N_SPARSE = 26
ONEHOT = 1000
FIELD = 39
K = 8
FEAT = N_DENSE + N_SPARSE * ONEHOT  # 26013
B = 4096
NCORES = 8
BC = B // NCORES  # 512 samples per core
P = 128
NT = BC // P  # 4 tiles per core
D = FIELD * K  # 312
DW = D + 1  # 313 (col 312 carries the linear weight)
E = 320  # gathered row width, padded so the 1280 B row is a multiple of 256
NI = N_SPARSE * P  # 3328 indices per tile gather
SQRT_HALF = 0.7071067811865476

_cached_nc = None


def _build_program():
    global _cached_nc
    if _cached_nc is not None:
        return _cached_nc

    import concourse.bacc as bacc
    import concourse.mybir as mybir
    from concourse.tile import TileContext
    from concourse import library_config

    nc = bacc.Bacc(
        "TRN2",
        debug=False,
        enable_asserts=False,
        target_bir_lowering=False,
        num_devices=NCORES,
        num_swdge_queues=4,
    )
    f32 = mybir.dt.float32
    i16 = mybir.dt.int16
    table = nc.dram_tensor("table", [FEAT, E], f32, kind="ExternalInput").ap()
    idx = nc.dram_tensor("idx", [P, NT * NI // 16], i16, kind="ExternalInput").ap()
    dnt = nc.dram_tensor("dnt", [N_DENSE + 1, BC], f32, kind="ExternalInput").ap()
    vdx = nc.dram_tensor("vdx", [N_DENSE + 1, DW], f32, kind="ExternalInput").ap()
    out = nc.dram_tensor("out", [BC, 1], f32, kind="ExternalOutput").ap()

    with TileContext(nc) as tc:
        with tc.tile_pool(name="const", bufs=1) as cpool, \
             tc.tile_pool(name="gath", bufs=NT) as gpool, \
             tc.tile_pool(name="work", bufs=2) as wpool, \
             tc.tile_pool(name="psum", bufs=2, space="PSUM") as ppool:
            nc.gpsimd.load_library(library_config.mlp)
            idx_sb = cpool.tile([P, NT * NI // 16], i16)
            nc.sync.dma_start(out=idx_sb[:], in_=idx[:])
            dnt_sb = cpool.tile([N_DENSE + 1, BC], f32)
            nc.sync.dma_start(out=dnt_sb[:], in_=dnt[:])
            vdx_sb = cpool.tile([N_DENSE + 1, DW], f32)
            nc.sync.dma_start(out=vdx_sb[:], in_=vdx[:])

            for t in range(NT):
                # gh[h][p, c, :] = table[flat_idx[t*128 + p, 13*h + c], :]
                # two half-gathers per tile on rotating SWDGE queues so the
                # adder tree can start on half 0 while half 1 streams in
                HNI = NI // 2  # 1664 indices per half
                HC = HNI // 16  # idx columns per half
                gh = []
                for h in range(2):
                    g = gpool.tile([P, 13 * E], f32, tag=f"g{h}")
                    g3 = g[:].rearrange("p (c e) -> p c e", e=E)
                    col = (2 * t + h) * HC
                    nc.gpsimd.dma_gather(
                        g3,
                        table[:],
                        idx_sb[:, col:col + HC],
                        HNI,
                        HNI,
                        E,
                        single_packet=False,
                        queue_num=(2 * t + h) % 4,
                    )
                    gh.append(g)
                # dense + w0 contribution: [128, 313]
                ps = ppool.tile([P, DW], f32, tag="ps")
                nc.tensor.matmul(
                    out=ps[:],
                    lhsT=dnt_sb[:, t * P:(t + 1) * P],
                    rhs=vdx_sb[:],
                    start=True,
                    stop=True,
                )
                # sum the 26 gathered rows with a contiguous pairwise tree
                # (fp32 tensor_tensor runs 1 elem/cycle; contiguous > strided)
                add = lambda o, a, b: nc.vector.tensor_tensor(
                    out=o, in0=a, in1=b, op=mybir.AluOpType.add
                )
                # in-place tree: each half collapses 13 blocks to 3, then 1
                for h in range(2):
                    g = gh[h]
                    add(g[:, :6 * E], g[:, :6 * E], g[:, 6 * E:12 * E])
                    add(g[:, :3 * E], g[:, :3 * E], g[:, 3 * E:6 * E])
                g0, g1 = gh
                add(g0[:, :3 * E], g0[:, :3 * E], g1[:, :3 * E])
                add(g0[:, :E], g0[:, :E], g0[:, E:2 * E])
                add(g0[:, :E], g0[:, :E], g0[:, 2 * E:3 * E])
                add(g0[:, :E], g0[:, :E], g0[:, 12 * E:13 * E])
                add(g0[:, :E], g0[:, :E], g1[:, 12 * E:13 * E])
                tot = wpool.tile([P, DW], f32, tag="tot")
                add(tot[:], g0[:, :DW], ps[:])
                # s_k = sum_i field_f[i, k]: view [P, 8, 39], reduce innermost
                s8 = wpool.tile([P, K], f32, tag="s8")
                tv = tot[:, :D].rearrange("p (i k) -> p k i", k=K)
                nc.vector.reduce_sum(out=s8[:], in_=tv, axis=mybir.AxisListType.X)
                # 0.5 * sum of squares, fused on the scalar engine
                sq = wpool.tile([P, D], f32, tag="sq")
                h_sumsq = wpool.tile([P, 1], f32, tag="h_sumsq")
                nc.scalar.activation(
                    out=sq[:], in_=tot[:, :D],
                    func=mybir.ActivationFunctionType.Square,
                    scale=SQRT_HALF, accum_out=h_sumsq[:],
                )
                sq8 = wpool.tile([P, K], f32, tag="sq8")
                h_ssq = wpool.tile([P, 1], f32, tag="h_ssq")
                nc.scalar.activation(
                    out=sq8[:], in_=s8[:],
                    func=mybir.ActivationFunctionType.Square,
                    scale=SQRT_HALF, accum_out=h_ssq[:],
                )
                ot = wpool.tile([P, 1], f32, tag="ot")
                nc.vector.tensor_tensor(
                    out=ot[:], in0=h_ssq[:], in1=h_sumsq[:],
                    op=mybir.AluOpType.subtract,
                )
                add(ot[:], ot[:], tot[:, D:DW])
                nc.sync.dma_start(out=out[t * P:(t + 1) * P, :], in_=ot[:])

    nc.compile()
    _cached_nc = nc
    return nc


def _prepare_inputs(inputs, w0, w, v):
    dense = np.ascontiguousarray(inputs[:, :N_DENSE].astype(np.float32))
    idx = inputs[:, N_DENSE:].astype(np.int32)
    flat_idx = (N_DENSE + np.arange(N_SPARSE, dtype=np.int32) * ONEHOT)[None, :] + idx

    table = np.zeros((FEAT, E), np.float32)
    table[:, :D] = v.reshape(FEAT, D)
    table[:, D] = np.asarray(w, np.float32).reshape(FEAT)
    w0_row = np.zeros((1, DW), np.float32)
    w0_row[0, D] = np.asarray(w0, np.float32).reshape(-1)[0]
    vdx = np.ascontiguousarray(np.concatenate([table[:N_DENSE, :DW], w0_row], axis=0))

    in_maps = []
    for c in range(NCORES):
        sl = slice(c * BC, (c + 1) * BC)
        dnt = np.concatenate(
            [dense[sl].T, np.ones((1, BC), np.float32)], axis=0
        )  # [14, 512]
        # per tile t the gather consumes indices i = c*128 + p, laid out
        # int16 at [i % 16, i // 16] in the first 16 partitions, replicated
        # 8x down the partitions (one copy per Q7 core)
        fi = flat_idx[sl].astype(np.int16)  # [512, 26]
        blocks = []
        for t in range(NT):
            for h in range(2):
                # half h covers fields 13h..13h+12; order i = c_local*128 + p
                lin = fi[t * P:(t + 1) * P, 13 * h:13 * (h + 1)].T.reshape(NI // 2)
                blk = lin.reshape(NI // 32, 16).T  # [16, HNI/16]
                blocks.append(np.tile(blk, (8, 1)))  # [128, HNI/16]
        idx_buf = np.ascontiguousarray(np.concatenate(blocks, axis=1))
        in_maps.append(
            {
                "table": table,
                "idx": idx_buf,
                "dnt": np.ascontiguousarray(dnt),
                "vdx": vdx,
            }
        )
    return in_maps


def kernel(**inputs):
    from concourse import bass_utils

    nc = _build_program()
    in_maps = _prepare_inputs(
        np.asarray(inputs["inputs"]),
        np.asarray(inputs["w0"]),
        np.asarray(inputs["w"]),
        np.asarray(inputs["v"]),
    )
    res = bass_utils.run_bass_kernel_spmd(nc, in_maps, core_ids=list(range(NCORES)))
    outs = [np.asarray(res.results[c]["out"]) for c in range(NCORES)]
    return np.concatenate(outs, axis=0).astype(np.float32)


# revision 14
# speedup vs baseline: 1.7832x; 1.0783x over previous
"""FFM layer kernel for Trainium2, data-parallel over batch on 8 NeuronCores.

The reference computes, for each sample b:
    x = [dense(13) | onehot(26 fields x 1000)]            # [B, 26013]
    linear = w0 + x @ w                                   # [B, 1]
    field_f = einsum('bf,fik->bik', x, v)                 # [B, 39, 8]
    inter = 0.5*((sum_i field_f)^2.sum(k) - (field_f^2).sum(i,k))
    out = linear + inter

Because x is one-hot in the sparse block, x @ [v|w] is a 26-row gather from
a [26013, 320] table (cols 0..311 = flattened v row, col 312 = w, 313.. pad)
plus a tiny dense [14]x[14,313] matmul (row 13 = ones row carrying w0 into
col 312).  Each core handles 512 samples as 4 tiles of 128; each tile's 26
rows/sample are fetched by one gpsimd dma_gather (3328 rows of 1280 B).
"""

import numpy as np

N_DENSE = 13
N_SPARSE = 26
ONEHOT = 1000
FIELD = 39
K = 8
FEAT = N_DENSE + N_SPARSE * ONEHOT  # 26013
B = 4096
NCORES = 8
BC = B // NCORES  # 512 samples per core
P = 128
NT = BC // P  # 4 tiles per core
D = FIELD * K  # 312
DW = D + 1  # 313 (col 312 carries the linear weight)
E = 384  # gathered fp16 row width, padded so the 768 B row is a multiple of 256
NI = N_SPARSE * P  # 3328 indices per tile gather
SQRT_HALF = 0.7071067811865476

_cached_nc = None


def _build_program():
    global _cached_nc
    if _cached_nc is not None:
        return _cached_nc

    import concourse.bacc as bacc
    import concourse.mybir as mybir
    from concourse.tile import TileContext
    from concourse import library_config

    nc = bacc.Bacc(
        "TRN2",
        debug=False,
        enable_asserts=False,
        target_bir_lowering=False,
        num_devices=NCORES,
        num_swdge_queues=4,
    )
    f32 = mybir.dt.float32
    f16 = mybir.dt.float16
    i16 = mybir.dt.int16
    table = nc.dram_tensor("table", [FEAT, E], f16, kind="ExternalInput").ap()
    idx = nc.dram_tensor("idx", [P, NT * NI // 16], i16, kind="ExternalInput").ap()
    dnt = nc.dram_tensor("dnt", [N_DENSE + 1, BC], f32, kind="ExternalInput").ap()
    vdx = nc.dram_tensor("vdx", [N_DENSE + 1, DW], f32, kind="ExternalInput").ap()
    out = nc.dram_tensor("out", [BC, 1], f32, kind="ExternalOutput").ap()

    with TileContext(nc) as tc:
        with tc.tile_pool(name="const", bufs=1) as cpool, \
             tc.tile_pool(name="gath", bufs=NT) as gpool, \
             tc.tile_pool(name="work", bufs=2) as wpool, \
             tc.tile_pool(name="psum", bufs=2, space="PSUM") as ppool:
            nc.gpsimd.load_library(library_config.mlp)
            idx_sb = cpool.tile([P, NT * NI // 16], i16)
            nc.sync.dma_start(out=idx_sb[:], in_=idx[:])
            dnt_sb = cpool.tile([N_DENSE + 1, BC], f32)
            nc.sync.dma_start(out=dnt_sb[:], in_=dnt[:])
            vdx_sb = cpool.tile([N_DENSE + 1, DW], f32)
            nc.sync.dma_start(out=vdx_sb[:], in_=vdx[:])

            for t in range(NT):
                # gh[h][p, c, :] = table[flat_idx[t*128 + p, 13*h + c], :]
                # two half-gathers per tile on rotating SWDGE queues so the
                # adder tree can start on half 0 while half 1 streams in
                HNI = NI // 2  # 1664 indices per half
                HC = HNI // 16  # idx columns per half
                gh = []
                for h in range(2):
                    g = gpool.tile([P, 13 * E], f16, tag=f"g{h}")
                    g3 = g[:].rearrange("p (c e) -> p c e", e=E)
                    col = (2 * t + h) * HC
                    nc.gpsimd.dma_gather(
                        g3,
                        table[:],
                        idx_sb[:, col:col + HC],
                        HNI,
                        HNI,
                        E,
                        single_packet=False,
                        queue_num=(2 * t + h) % 4,
                    )
                    gh.append(g)
                # dense + w0 contribution: [128, 313]
                ps = ppool.tile([P, DW], f32, tag="ps")
                nc.tensor.matmul(
                    out=ps[:],
                    lhsT=dnt_sb[:, t * P:(t + 1) * P],
                    rhs=vdx_sb[:],
                    start=True,
                    stop=True,
                )
                # sum the 26 gathered rows with a contiguous pairwise tree
                # (fp32 tensor_tensor runs 1 elem/cycle; contiguous > strided)
                add = lambda o, a, b: nc.vector.tensor_tensor(
                    out=o, in0=a, in1=b, op=mybir.AluOpType.add
                )
                # fp16 halves collapse 13 blocks -> fp32 partials -> 1 block
                a6 = []
                for h in range(2):
                    g = gh[h]
                    a = wpool.tile([P, 6 * E], f32, tag=f"a6_{h}")
                    add(a[:], g[:, :6 * E], g[:, 6 * E:12 * E])
                    add(a[:, :3 * E], a[:, :3 * E], a[:, 3 * E:6 * E])
                    a6.append(a)
                a0, a1 = a6
                add(a0[:, :3 * E], a0[:, :3 * E], a1[:, :3 * E])
                add(a0[:, :E], a0[:, :E], a0[:, E:2 * E])
                add(a0[:, :E], a0[:, :E], a0[:, 2 * E:3 * E])
                add(a0[:, :E], a0[:, :E], gh[0][:, 12 * E:13 * E])
                add(a0[:, :E], a0[:, :E], gh[1][:, 12 * E:13 * E])
                tot = wpool.tile([P, DW], f32, tag="tot")
                add(tot[:], a0[:, :DW], ps[:])
                # s_k = sum_i field_f[i, k]: view [P, 8, 39], reduce innermost
                s8 = wpool.tile([P, K], f32, tag="s8")
                tv = tot[:, :D].rearrange("p (i k) -> p k i", k=K)
                nc.vector.reduce_sum(out=s8[:], in_=tv, axis=mybir.AxisListType.X)
                # 0.5 * sum of squares, fused on the scalar engine
                sq = wpool.tile([P, D], f32, tag="sq")
                h_sumsq = wpool.tile([P, 1], f32, tag="h_sumsq")
                nc.scalar.activation(
                    out=sq[:], in_=tot[:, :D],
                    func=mybir.ActivationFunctionType.Square,
                    scale=SQRT_HALF, accum_out=h_sumsq[:],
                )
                sq8 = wpool.tile([P, K], f32, tag="sq8")
                h_ssq = wpool.tile([P, 1], f32, tag="h_ssq")
                nc.scalar.activation(
                    out=sq8[:], in_=s8[:],
                    func=mybir.ActivationFunctionType.Square,
                    scale=SQRT_HALF, accum_out=h_ssq[:],
                )
                ot = wpool.tile([P, 1], f32, tag="ot")
                nc.vector.tensor_tensor(
                    out=ot[:], in0=h_ssq[:], in1=h_sumsq[:],
                    op=mybir.AluOpType.subtract,
                )
                add(ot[:], ot[:], tot[:, D:DW])
                nc.sync.dma_start(out=out[t * P:(t + 1) * P, :], in_=ot[:])

    nc.compile()
    _cached_nc = nc
    return nc


def _prepare_inputs(inputs, w0, w, v):
    dense = np.ascontiguousarray(inputs[:, :N_DENSE].astype(np.float32))
    idx = inputs[:, N_DENSE:].astype(np.int32)
    flat_idx = (N_DENSE + np.arange(N_SPARSE, dtype=np.int32) * ONEHOT)[None, :] + idx

    table = np.zeros((FEAT, E), np.float16)
    table[:, :D] = v.reshape(FEAT, D).astype(np.float16)
    table[:, D] = np.asarray(w, np.float32).reshape(FEAT).astype(np.float16)
    w0_row = np.zeros((1, DW), np.float32)
    w0_row[0, D] = np.asarray(w0, np.float32).reshape(-1)[0]
    vdx_top = np.concatenate(
        [v.reshape(FEAT, D)[:N_DENSE], np.asarray(w, np.float32).reshape(FEAT, 1)[:N_DENSE]],
        axis=1,
    ).astype(np.float32)
    vdx = np.ascontiguousarray(np.concatenate([vdx_top, w0_row], axis=0))

    in_maps = []
    for c in range(NCORES):
        sl = slice(c * BC, (c + 1) * BC)
        dnt = np.concatenate(
            [dense[sl].T, np.ones((1, BC), np.float32)], axis=0
        )  # [14, 512]
        # per tile t the gather consumes indices i = c*128 + p, laid out
        # int16 at [i % 16, i // 16] in the first 16 partitions, replicated
        # 8x down the partitions (one copy per Q7 core)
        fi = flat_idx[sl].astype(np.int16)  # [512, 26]
        blocks = []
        for t in range(NT):
            for h in range(2):
                # half h covers fields 13h..13h+12; order i = c_local*128 + p
                lin = fi[t * P:(t + 1) * P, 13 * h:13 * (h + 1)].T.reshape(NI // 2)
                blk = lin.reshape(NI // 32, 16).T  # [16, HNI/16]
                blocks.append(np.tile(blk, (8, 1)))  # [128, HNI/16]
        idx_buf = np.ascontiguousarray(np.concatenate(blocks, axis=1))
        in_maps.append(
            {
                "table": table,
                "idx": idx_buf,
                "dnt": np.ascontiguousarray(dnt),
                "vdx": vdx,
            }
        )
    return in_maps


def kernel(**inputs):
    from concourse import bass_utils

    nc = _build_program()
    in_maps = _prepare_inputs(
        np.asarray(inputs["inputs"]),
        np.asarray(inputs["w0"]),
        np.asarray(inputs["w"]),
        np.asarray(inputs["v"]),
    )
    res = bass_utils.run_bass_kernel_spmd(nc, in_maps, core_ids=list(range(NCORES)))
    outs = [np.asarray(res.results[c]["out"]) for c in range(NCORES)]
    return np.concatenate(outs, axis=0).astype(np.float32)


# revision 15
# speedup vs baseline: 1.8921x; 1.0610x over previous
"""FFM layer kernel for Trainium2, data-parallel over batch on 8 NeuronCores.

The reference computes, for each sample b:
    x = [dense(13) | onehot(26 fields x 1000)]            # [B, 26013]
    linear = w0 + x @ w                                   # [B, 1]
    field_f = einsum('bf,fik->bik', x, v)                 # [B, 39, 8]
    inter = 0.5*((sum_i field_f)^2.sum(k) - (field_f^2).sum(i,k))
    out = linear + inter

Because x is one-hot in the sparse block, x @ [v|w] is a 26-row gather from
a [26013, 320] table (cols 0..311 = flattened v row, col 312 = w, 313.. pad)
plus a tiny dense [14]x[14,313] matmul (row 13 = ones row carrying w0 into
col 312).  Each core handles 512 samples as 4 tiles of 128; each tile's 26
rows/sample are fetched by one gpsimd dma_gather (3328 rows of 1280 B).
"""

import numpy as np

N_DENSE = 13
N_SPARSE = 26
ONEHOT = 1000
FIELD = 39
K = 8
FEAT = N_DENSE + N_SPARSE * ONEHOT  # 26013
B = 4096
NCORES = 8
BC = B // NCORES  # 512 samples per core
P = 128
NT = BC // P  # 4 tiles per core
D = FIELD * K  # 312
DW = D + 1  # 313 (col 312 carries the linear weight)
E = 384  # gathered fp16 row width, padded so the 768 B row is a multiple of 256
NI = N_SPARSE * P  # 3328 indices per tile gather
SQRT_HALF = 0.7071067811865476

_cached_nc = None


def _build_program():
    global _cached_nc
    if _cached_nc is not None:
        return _cached_nc

    import concourse.bacc as bacc
    import concourse.mybir as mybir
    from concourse.tile import TileContext
    from concourse import library_config

    nc = bacc.Bacc(
        "TRN2",
        debug=False,
        enable_asserts=False,
        target_bir_lowering=False,
        num_devices=NCORES,
        num_swdge_queues=4,
    )
    f32 = mybir.dt.float32
    f16 = mybir.dt.float16
    i16 = mybir.dt.int16
    table = nc.dram_tensor("table", [FEAT, E], f16, kind="ExternalInput").ap()
    idx = nc.dram_tensor("idx", [P, NT * NI // 16], i16, kind="ExternalInput").ap()
    dnt = nc.dram_tensor("dnt", [N_DENSE + 1, BC], f32, kind="ExternalInput").ap()
    vdx = nc.dram_tensor("vdx", [N_DENSE + 1, DW], f32, kind="ExternalInput").ap()
    out = nc.dram_tensor("out", [BC, 1], f32, kind="ExternalOutput").ap()

    with TileContext(nc) as tc:
        with tc.tile_pool(name="const", bufs=1) as cpool, \
             tc.tile_pool(name="gath", bufs=NT) as gpool, \
             tc.tile_pool(name="work", bufs=2) as wpool, \
             tc.tile_pool(name="psum", bufs=2, space="PSUM") as ppool:
            nc.gpsimd.load_library(library_config.mlp)
            idx_sb = cpool.tile([P, NT * NI // 16], i16)
            nc.sync.dma_start(out=idx_sb[:], in_=idx[:])
            dnt_sb = cpool.tile([N_DENSE + 1, BC], f32)
            nc.sync.dma_start(out=dnt_sb[:], in_=dnt[:])
            vdx_sb = cpool.tile([N_DENSE + 1, DW], f32)
            nc.sync.dma_start(out=vdx_sb[:], in_=vdx[:])

            for t in range(NT):
                # gh[h][p, c, :] = table[flat_idx[t*128 + p, 13*h + c], :]
                # two half-gathers per tile on rotating SWDGE queues so the
                # adder tree can start on half 0 while half 1 streams in
                HNI = NI // 2  # 1664 indices per half
                HC = HNI // 16  # idx columns per half
                gh = []
                for h in range(2):
                    g = gpool.tile([P, 13 * E], f16, tag=f"g{h}")
                    g3 = g[:].rearrange("p (c e) -> p c e", e=E)
                    col = (2 * t + h) * HC
                    nc.gpsimd.dma_gather(
                        g3,
                        table[:],
                        idx_sb[:, col:col + HC],
                        HNI,
                        HNI,
                        E,
                        single_packet=False,
                        queue_num=(2 * t + h) % 4,
                    )
                    gh.append(g)
                # dense + w0 contribution: [128, 313]
                ps = ppool.tile([P, DW], f32, tag="ps")
                nc.tensor.matmul(
                    out=ps[:],
                    lhsT=dnt_sb[:, t * P:(t + 1) * P],
                    rhs=vdx_sb[:],
                    start=True,
                    stop=True,
                )
                # sum the 26 gathered rows with a contiguous pairwise tree
                # (fp32 tensor_tensor runs 1 elem/cycle; contiguous > strided)
                add = lambda o, a, b: nc.vector.tensor_tensor(
                    out=o, in0=a, in1=b, op=mybir.AluOpType.add
                )
                # fp16 halves collapse 13 blocks -> fp32 partials -> 1 block;
                # all adds use 313-wide views so the 71 pad columns per block
                # are never touched by the vector engine
                W = DW  # 313 payload columns per 384-wide block
                a6 = []
                for h in range(2):
                    g3 = gh[h][:].rearrange("p (c e) -> p c e", e=E)
                    a = wpool.tile([P, 6 * E], f32, tag=f"a6_{h}")
                    a3 = a[:].rearrange("p (c e) -> p c e", e=E)
                    add(a3[:, 0:6, 0:W], g3[:, 0:6, 0:W], g3[:, 6:12, 0:W])
                    add(a3[:, 0:3, 0:W], a3[:, 0:3, 0:W], a3[:, 3:6, 0:W])
                    a6.append(a)
                a03 = a6[0][:].rearrange("p (c e) -> p c e", e=E)
                a13 = a6[1][:].rearrange("p (c e) -> p c e", e=E)
                g03 = gh[0][:].rearrange("p (c e) -> p c e", e=E)
                g13 = gh[1][:].rearrange("p (c e) -> p c e", e=E)
                add(a03[:, 0:3, 0:W], a03[:, 0:3, 0:W], a13[:, 0:3, 0:W])
                add(a03[:, 0, 0:W], a03[:, 0, 0:W], a03[:, 1, 0:W])
                add(a03[:, 0, 0:W], a03[:, 0, 0:W], a03[:, 2, 0:W])
                add(a03[:, 0, 0:W], a03[:, 0, 0:W], g03[:, 12, 0:W])
                add(a03[:, 0, 0:W], a03[:, 0, 0:W], g13[:, 12, 0:W])
                tot = wpool.tile([P, DW], f32, tag="tot")
                add(tot[:], a6[0][:, :DW], ps[:])
                # s_k = sum_i field_f[i, k]: view [P, 8, 39], reduce innermost
                s8 = wpool.tile([P, K], f32, tag="s8")
                tv = tot[:, :D].rearrange("p (i k) -> p k i", k=K)
                nc.vector.reduce_sum(out=s8[:], in_=tv, axis=mybir.AxisListType.X)
                # 0.5 * sum of squares, fused on the scalar engine
                sq = wpool.tile([P, D], f32, tag="sq")
                h_sumsq = wpool.tile([P, 1], f32, tag="h_sumsq")
                nc.scalar.activation(
                    out=sq[:], in_=tot[:, :D],
                    func=mybir.ActivationFunctionType.Square,
                    scale=SQRT_HALF, accum_out=h_sumsq[:],
                )
                sq8 = wpool.tile([P, K], f32, tag="sq8")
                h_ssq = wpool.tile([P, 1], f32, tag="h_ssq")
                nc.scalar.activation(
                    out=sq8[:], in_=s8[:],
                    func=mybir.ActivationFunctionType.Square,
                    scale=SQRT_HALF, accum_out=h_ssq[:],
                )
                ot = wpool.tile([P, 1], f32, tag="ot")
                nc.vector.tensor_tensor(
                    out=ot[:], in0=h_ssq[:], in1=h_sumsq[:],
                    op=mybir.AluOpType.subtract,
                )
                add(ot[:], ot[:], tot[:, D:DW])
                nc.sync.dma_start(out=out[t * P:(t + 1) * P, :], in_=ot[:])

    nc.compile()
    _cached_nc = nc
    return nc


def _prepare_inputs(inputs, w0, w, v):
    dense = np.ascontiguousarray(inputs[:, :N_DENSE].astype(np.float32))
    idx = inputs[:, N_DENSE:].astype(np.int32)
    flat_idx = (N_DENSE + np.arange(N_SPARSE, dtype=np.int32) * ONEHOT)[None, :] + idx

    table = np.zeros((FEAT, E), np.float16)
    table[:, :D] = v.reshape(FEAT, D).astype(np.float16)
    table[:, D] = np.asarray(w, np.float32).reshape(FEAT).astype(np.float16)
    w0_row = np.zeros((1, DW), np.float32)
    w0_row[0, D] = np.asarray(w0, np.float32).reshape(-1)[0]
    vdx_top = np.concatenate(
        [v.reshape(FEAT, D)[:N_DENSE], np.asarray(w, np.float32).reshape(FEAT, 1)[:N_DENSE]],
        axis=1,
    ).astype(np.float32)
    vdx = np.ascontiguousarray(np.concatenate([vdx_top, w0_row], axis=0))

    in_maps = []
    for c in range(NCORES):
        sl = slice(c * BC, (c + 1) * BC)
        dnt = np.concatenate(
            [dense[sl].T, np.ones((1, BC), np.float32)], axis=0
        )  # [14, 512]
        # per tile t the gather consumes indices i = c*128 + p, laid out
        # int16 at [i % 16, i // 16] in the first 16 partitions, replicated
        # 8x down the partitions (one copy per Q7 core)
        fi = flat_idx[sl].astype(np.int16)  # [512, 26]
        blocks = []
        for t in range(NT):
            for h in range(2):
                # half h covers fields 13h..13h+12; order i = c_local*128 + p
                lin = fi[t * P:(t + 1) * P, 13 * h:13 * (h + 1)].T.reshape(NI // 2)
                blk = lin.reshape(NI // 32, 16).T  # [16, HNI/16]
                blocks.append(np.tile(blk, (8, 1)))  # [128, HNI/16]
        idx_buf = np.ascontiguousarray(np.concatenate(blocks, axis=1))
        in_maps.append(
            {
                "table": table,
                "idx": idx_buf,
                "dnt": np.ascontiguousarray(dnt),
                "vdx": vdx,
            }
        )
    return in_maps


def kernel(**inputs):
    from concourse import bass_utils

    nc = _build_program()
    in_maps = _prepare_inputs(
        np.asarray(inputs["inputs"]),
        np.asarray(inputs["w0"]),
        np.asarray(inputs["w"]),
        np.asarray(inputs["v"]),
    )
    res = bass_utils.run_bass_kernel_spmd(nc, in_maps, core_ids=list(range(NCORES)))
    outs = [np.asarray(res.results[c]["out"]) for c in range(NCORES)]
    return np.concatenate(outs, axis=0).astype(np.float32)


# revision 16
# speedup vs baseline: 1.9213x; 1.0155x over previous
"""FFM layer kernel for Trainium2, data-parallel over batch on 8 NeuronCores.

The reference computes, for each sample b:
    x = [dense(13) | onehot(26 fields x 1000)]            # [B, 26013]
    linear = w0 + x @ w                                   # [B, 1]
    field_f = einsum('bf,fik->bik', x, v)                 # [B, 39, 8]
    inter = 0.5*((sum_i field_f)^2.sum(k) - (field_f^2).sum(i,k))
    out = linear + inter

Because x is one-hot in the sparse block, x @ [v|w] is a 26-row gather from
a [26013, 320] table (cols 0..311 = flattened v row, col 312 = w, 313.. pad)
plus a tiny dense [14]x[14,313] matmul (row 13 = ones row carrying w0 into
col 312).  Each core handles 512 samples as 4 tiles of 128; each tile's 26
rows/sample are fetched by one gpsimd dma_gather (3328 rows of 1280 B).
"""

import numpy as np

N_DENSE = 13
N_SPARSE = 26
ONEHOT = 1000
FIELD = 39
K = 8
FEAT = N_DENSE + N_SPARSE * ONEHOT  # 26013
B = 4096
NCORES = 8
BC = B // NCORES  # 512 samples per core
P = 128
NT = BC // P  # 4 tiles per core
D = FIELD * K  # 312
DW = D + 1  # 313 (col 312 carries the linear weight)
E = 384  # gathered fp16 row width, padded so the 768 B row is a multiple of 256
NI = N_SPARSE * P  # 3328 indices per tile gather
SQRT_HALF = 0.7071067811865476

_cached_nc = None


def _build_program():
    global _cached_nc
    if _cached_nc is not None:
        return _cached_nc

    import concourse.bacc as bacc
    import concourse.mybir as mybir
    from concourse.tile import TileContext
    from concourse import library_config

    nc = bacc.Bacc(
        "TRN2",
        debug=False,
        enable_asserts=False,
        target_bir_lowering=False,
        num_devices=NCORES,
        num_swdge_queues=4,
        dynamic_dma_scratch_size=32768,
    )
    f32 = mybir.dt.float32
    f16 = mybir.dt.float16
    i16 = mybir.dt.int16
    table = nc.dram_tensor("table", [FEAT, E], f16, kind="ExternalInput").ap()
    idx = nc.dram_tensor("idx", [P, NT * NI // 16], i16, kind="ExternalInput").ap()
    dnt = nc.dram_tensor("dnt", [N_DENSE + 1, BC], f32, kind="ExternalInput").ap()
    vdx = nc.dram_tensor("vdx", [N_DENSE + 1, DW], f32, kind="ExternalInput").ap()
    out = nc.dram_tensor("out", [BC, 1], f32, kind="ExternalOutput").ap()

    with TileContext(nc) as tc:
        with tc.tile_pool(name="const", bufs=1) as cpool, \
             tc.tile_pool(name="gath", bufs=NT) as gpool, \
             tc.tile_pool(name="work", bufs=2) as wpool, \
             tc.tile_pool(name="psum", bufs=2, space="PSUM") as ppool:
            nc.gpsimd.load_library(library_config.mlp)
            idx_sb = cpool.tile([P, NT * NI // 16], i16)
            nc.sync.dma_start(out=idx_sb[:], in_=idx[:])
            dnt_sb = cpool.tile([N_DENSE + 1, BC], f32)
            nc.sync.dma_start(out=dnt_sb[:], in_=dnt[:])
            vdx_sb = cpool.tile([N_DENSE + 1, DW], f32)
            nc.sync.dma_start(out=vdx_sb[:], in_=vdx[:])

            for t in range(NT):
                # gh[h][p, c, :] = table[flat_idx[t*128 + p, 13*h + c], :]
                # two half-gathers per tile on rotating SWDGE queues so the
                # adder tree can start on half 0 while half 1 streams in
                HNI = NI // 2  # 1664 indices per half
                HC = HNI // 16  # idx columns per half
                gh = []
                for h in range(2):
                    g = gpool.tile([P, 13 * E], f16, tag=f"g{h}")
                    g3 = g[:].rearrange("p (c e) -> p c e", e=E)
                    col = (2 * t + h) * HC
                    nc.gpsimd.dma_gather(
                        g3,
                        table[:],
                        idx_sb[:, col:col + HC],
                        HNI,
                        HNI,
                        E,
                        single_packet=False,
                        queue_num=(2 * t + h) % 4,
                    )
                    gh.append(g)
                # dense + w0 contribution: [128, 313]
                ps = ppool.tile([P, DW], f32, tag="ps")
                nc.tensor.matmul(
                    out=ps[:],
                    lhsT=dnt_sb[:, t * P:(t + 1) * P],
                    rhs=vdx_sb[:],
                    start=True,
                    stop=True,
                )
                # sum the 26 gathered rows with a contiguous pairwise tree
                # (fp32 tensor_tensor runs 1 elem/cycle; contiguous > strided)
                add = lambda o, a, b: nc.vector.tensor_tensor(
                    out=o, in0=a, in1=b, op=mybir.AluOpType.add
                )
                # fp16 halves collapse 13 blocks -> fp32 partials -> 1 block;
                # all adds use 313-wide views so the 71 pad columns per block
                # are never touched by the vector engine
                W = DW  # 313 payload columns per 384-wide block
                a6 = []
                for h in range(2):
                    g3 = gh[h][:].rearrange("p (c e) -> p c e", e=E)
                    a = wpool.tile([P, 6 * E], f32, tag=f"a6_{h}")
                    a3 = a[:].rearrange("p (c e) -> p c e", e=E)
                    add(a3[:, 0:6, 0:W], g3[:, 0:6, 0:W], g3[:, 6:12, 0:W])
                    add(a3[:, 0:3, 0:W], a3[:, 0:3, 0:W], a3[:, 3:6, 0:W])
                    a6.append(a)
                a03 = a6[0][:].rearrange("p (c e) -> p c e", e=E)
                a13 = a6[1][:].rearrange("p (c e) -> p c e", e=E)
                g03 = gh[0][:].rearrange("p (c e) -> p c e", e=E)
                g13 = gh[1][:].rearrange("p (c e) -> p c e", e=E)
                add(a03[:, 0:3, 0:W], a03[:, 0:3, 0:W], a13[:, 0:3, 0:W])
                add(a03[:, 0, 0:W], a03[:, 0, 0:W], a03[:, 1, 0:W])
                add(a03[:, 0, 0:W], a03[:, 0, 0:W], a03[:, 2, 0:W])
                add(a03[:, 0, 0:W], a03[:, 0, 0:W], g03[:, 12, 0:W])
                add(a03[:, 0, 0:W], a03[:, 0, 0:W], g13[:, 12, 0:W])
                tot = wpool.tile([P, DW], f32, tag="tot")
                add(tot[:], a6[0][:, :DW], ps[:])
                # s_k = sum_i field_f[i, k]: view [P, 8, 39], reduce innermost
                s8 = wpool.tile([P, K], f32, tag="s8")
                tv = tot[:, :D].rearrange("p (i k) -> p k i", k=K)
                nc.vector.reduce_sum(out=s8[:], in_=tv, axis=mybir.AxisListType.X)
                # 0.5 * sum of squares, fused on the scalar engine
                sq = wpool.tile([P, D], f32, tag="sq")
                h_sumsq = wpool.tile([P, 1], f32, tag="h_sumsq")
                nc.scalar.activation(
                    out=sq[:], in_=tot[:, :D],
                    func=mybir.ActivationFunctionType.Square,
                    scale=SQRT_HALF, accum_out=h_sumsq[:],
                )
                sq8 = wpool.tile([P, K], f32, tag="sq8")
                h_ssq = wpool.tile([P, 1], f32, tag="h_ssq")
                nc.scalar.activation(
                    out=sq8[:], in_=s8[:],
                    func=mybir.ActivationFunctionType.Square,
                    scale=SQRT_HALF, accum_out=h_ssq[:],
                )
                ot = wpool.tile([P, 1], f32, tag="ot")
                nc.vector.tensor_tensor(
                    out=ot[:], in0=h_ssq[:], in1=h_sumsq[:],
                    op=mybir.AluOpType.subtract,
                )
                add(ot[:], ot[:], tot[:, D:DW])
                nc.sync.dma_start(out=out[t * P:(t + 1) * P, :], in_=ot[:])

    nc.compile()
    _cached_nc = nc
    return nc


def _prepare_inputs(inputs, w0, w, v):
    dense = np.ascontiguousarray(inputs[:, :N_DENSE].astype(np.float32))
    idx = inputs[:, N_DENSE:].astype(np.int32)
    flat_idx = (N_DENSE + np.arange(N_SPARSE, dtype=np.int32) * ONEHOT)[None, :] + idx

    table = np.zeros((FEAT, E), np.float16)
    table[:, :D] = v.reshape(FEAT, D).astype(np.float16)
    table[:, D] = np.asarray(w, np.float32).reshape(FEAT).astype(np.float16)
    w0_row = np.zeros((1, DW), np.float32)
    w0_row[0, D] = np.asarray(w0, np.float32).reshape(-1)[0]
    vdx_top = np.concatenate(
        [v.reshape(FEAT, D)[:N_DENSE], np.asarray(w, np.float32).reshape(FEAT, 1)[:N_DENSE]],
        axis=1,
    ).astype(np.float32)
    vdx = np.ascontiguousarray(np.concatenate([vdx_top, w0_row], axis=0))

    in_maps = []
    for c in range(NCORES):
        sl = slice(c * BC, (c + 1) * BC)
        dnt = np.concatenate(
            [dense[sl].T, np.ones((1, BC), np.float32)], axis=0
        )  # [14, 512]
        # per tile t the gather consumes indices i = c*128 + p, laid out
        # int16 at [i % 16, i // 16] in the first 16 partitions, replicated
        # 8x down the partitions (one copy per Q7 core)
        fi = flat_idx[sl].astype(np.int16)  # [512, 26]
        blocks = []
        for t in range(NT):
            for h in range(2):
                # half h covers fields 13h..13h+12; order i = c_local*128 + p
                lin = fi[t * P:(t + 1) * P, 13 * h:13 * (h + 1)].T.reshape(NI // 2)
                blk = lin.reshape(NI // 32, 16).T  # [16, HNI/16]
                blocks.append(np.tile(blk, (8, 1)))  # [128, HNI/16]
        idx_buf = np.ascontiguousarray(np.concatenate(blocks, axis=1))
        in_maps.append(
            {
                "table": table,
                "idx": idx_buf,
                "dnt": np.ascontiguousarray(dnt),
                "vdx": vdx,
            }
        )
    return in_maps


def kernel(**inputs):
    from concourse import bass_utils

    nc = _build_program()
    in_maps = _prepare_inputs(
        np.asarray(inputs["inputs"]),
        np.asarray(inputs["w0"]),
        np.asarray(inputs["w"]),
        np.asarray(inputs["v"]),
    )
    res = bass_utils.run_bass_kernel_spmd(nc, in_maps, core_ids=list(range(NCORES)))
    outs = [np.asarray(res.results[c]["out"]) for c in range(NCORES)]
    return np.concatenate(outs, axis=0).astype(np.float32)


# revision 17
# speedup vs baseline: 2.1194x; 1.1031x over previous
"""FFM layer kernel for Trainium2, data-parallel over batch on 8 NeuronCores.

The reference computes, for each sample b:
    x = [dense(13) | onehot(26 fields x 1000)]            # [B, 26013]
    linear = w0 + x @ w                                   # [B, 1]
    field_f = einsum('bf,fik->bik', x, v)                 # [B, 39, 8]
    inter = 0.5*((sum_i field_f)^2.sum(k) - (field_f^2).sum(i,k))
    out = linear + inter

Because x is one-hot in the sparse block, x @ [v|w] is a 26-row gather from
a [26013, 320] table (cols 0..311 = flattened v row, col 312 = w, 313.. pad)
plus a tiny dense [14]x[14,313] matmul (row 13 = ones row carrying w0 into
col 312).  Each core handles 512 samples as 4 tiles of 128; each tile's 26
rows/sample are fetched by one gpsimd dma_gather (3328 rows of 1280 B).
"""

import numpy as np

N_DENSE = 13
N_SPARSE = 26
ONEHOT = 1000
FIELD = 39
K = 8
FEAT = N_DENSE + N_SPARSE * ONEHOT  # 26013
B = 4096
NCORES = 8
BC = B // NCORES  # 512 samples per core
P = 128
NT = BC // P  # 4 tiles per core
D = FIELD * K  # 312
DW = D + 1  # 313 (col 312 carries the linear weight)
E = 384  # gathered fp16 row width, padded so the 768 B row is a multiple of 256
NI = N_SPARSE * P  # 3328 indices per tile gather
SQRT_HALF = 0.7071067811865476

_cached_nc = None


def _build_program():
    global _cached_nc
    if _cached_nc is not None:
        return _cached_nc

    import concourse.bacc as bacc
    import concourse.mybir as mybir
    from concourse.tile import TileContext
    from concourse import library_config

    nc = bacc.Bacc(
        "TRN2",
        debug=False,
        enable_asserts=False,
        target_bir_lowering=False,
        num_devices=NCORES,
        num_swdge_queues=4,
        dynamic_dma_scratch_size=32768,
    )
    f32 = mybir.dt.float32
    f16 = mybir.dt.float16
    i16 = mybir.dt.int16
    table = nc.dram_tensor("table", [FEAT, E], f16, kind="ExternalInput").ap()
    idx = nc.dram_tensor("idx", [P, NT * NI // 16], i16, kind="ExternalInput").ap()
    dnt = nc.dram_tensor("dnt", [N_DENSE + 1, BC], f32, kind="ExternalInput").ap()
    vdx = nc.dram_tensor("vdx", [N_DENSE + 1, DW], f32, kind="ExternalInput").ap()
    out = nc.dram_tensor("out", [BC, 1], f32, kind="ExternalOutput").ap()

    with TileContext(nc) as tc:
        with tc.tile_pool(name="const", bufs=1) as cpool, \
             tc.tile_pool(name="gath", bufs=NT) as gpool, \
             tc.tile_pool(name="work", bufs=2) as wpool, \
             tc.tile_pool(name="psum", bufs=2, space="PSUM") as ppool:
            nc.gpsimd.load_library(library_config.mlp)
            idx_sb = cpool.tile([P, NT * NI // 16], i16)
            nc.sync.dma_start(out=idx_sb[:], in_=idx[:])
            dnt_sb = cpool.tile([N_DENSE + 1, BC], f32)
            nc.sync.dma_start(out=dnt_sb[:], in_=dnt[:])
            vdx_sb = cpool.tile([N_DENSE + 1, DW], f32)
            nc.sync.dma_start(out=vdx_sb[:], in_=vdx[:])

            qn = 0
            for t in range(NT):
                # gh[h] covers 13 fields as two sub-gathers (7+6 fields) on
                # rotating SWDGE queues for finer stream interleaving
                HC = (NI // 2) // 16  # idx columns per 13-field half
                gh = []
                for h in range(2):
                    g = gpool.tile([P, 13 * E], f16, tag=f"g{h}")
                    g3 = g[:].rearrange("p (c e) -> p c e", e=E)
                    col = (2 * t + h) * HC
                    for (c0, nf) in ((0, 7), (7, 6)):
                        ni = nf * P
                        nc.gpsimd.dma_gather(
                            g3[:, c0:c0 + nf, :],
                            table[:],
                            idx_sb[:, col + c0 * 8:col + (c0 + nf) * 8],
                            ni,
                            ni,
                            E,
                            single_packet=False,
                            queue_num=qn % 4,
                        )
                        qn += 1
                    gh.append(g)
                # dense + w0 contribution: [128, 313]
                ps = ppool.tile([P, DW], f32, tag="ps")
                nc.tensor.matmul(
                    out=ps[:],
                    lhsT=dnt_sb[:, t * P:(t + 1) * P],
                    rhs=vdx_sb[:],
                    start=True,
                    stop=True,
                )
                # sum the 26 gathered rows with a contiguous pairwise tree
                # (fp32 tensor_tensor runs 1 elem/cycle; contiguous > strided)
                add = lambda o, a, b: nc.vector.tensor_tensor(
                    out=o, in0=a, in1=b, op=mybir.AluOpType.add
                )
                # fp16 halves collapse 13 blocks -> fp32 partials -> 1 block;
                # all adds use 313-wide views so the 71 pad columns per block
                # are never touched by the vector engine
                W = DW  # 313 payload columns per 384-wide block
                a6 = []
                for h in range(2):
                    g3 = gh[h][:].rearrange("p (c e) -> p c e", e=E)
                    a = wpool.tile([P, 6 * E], f32, tag=f"a6_{h}")
                    a3 = a[:].rearrange("p (c e) -> p c e", e=E)
                    add(a3[:, 0:6, 0:W], g3[:, 0:6, 0:W], g3[:, 6:12, 0:W])
                    add(a3[:, 0:3, 0:W], a3[:, 0:3, 0:W], a3[:, 3:6, 0:W])
                    a6.append(a)
                a03 = a6[0][:].rearrange("p (c e) -> p c e", e=E)
                a13 = a6[1][:].rearrange("p (c e) -> p c e", e=E)
                g03 = gh[0][:].rearrange("p (c e) -> p c e", e=E)
                g13 = gh[1][:].rearrange("p (c e) -> p c e", e=E)
                add(a03[:, 0:3, 0:W], a03[:, 0:3, 0:W], a13[:, 0:3, 0:W])
                add(a03[:, 0, 0:W], a03[:, 0, 0:W], a03[:, 1, 0:W])
                add(a03[:, 0, 0:W], a03[:, 0, 0:W], a03[:, 2, 0:W])
                add(a03[:, 0, 0:W], a03[:, 0, 0:W], g03[:, 12, 0:W])
                add(a03[:, 0, 0:W], a03[:, 0, 0:W], g13[:, 12, 0:W])
                tot = wpool.tile([P, DW], f32, tag="tot")
                add(tot[:], a6[0][:, :DW], ps[:])
                # s_k = sum_i field_f[i, k]: view [P, 8, 39], reduce innermost
                s8 = wpool.tile([P, K], f32, tag="s8")
                tv = tot[:, :D].rearrange("p (i k) -> p k i", k=K)
                nc.vector.reduce_sum(out=s8[:], in_=tv, axis=mybir.AxisListType.X)
                # 0.5 * sum of squares, fused on the scalar engine
                sq = wpool.tile([P, D], f32, tag="sq")
                h_sumsq = wpool.tile([P, 1], f32, tag="h_sumsq")
                nc.scalar.activation(
                    out=sq[:], in_=tot[:, :D],
                    func=mybir.ActivationFunctionType.Square,
                    scale=SQRT_HALF, accum_out=h_sumsq[:],
                )
                sq8 = wpool.tile([P, K], f32, tag="sq8")
                h_ssq = wpool.tile([P, 1], f32, tag="h_ssq")
                nc.scalar.activation(
                    out=sq8[:], in_=s8[:],
                    func=mybir.ActivationFunctionType.Square,
                    scale=SQRT_HALF, accum_out=h_ssq[:],
                )
                ot = wpool.tile([P, 1], f32, tag="ot")
                nc.vector.tensor_tensor(
                    out=ot[:], in0=h_ssq[:], in1=h_sumsq[:],
                    op=mybir.AluOpType.subtract,
                )
                add(ot[:], ot[:], tot[:, D:DW])
                nc.sync.dma_start(out=out[t * P:(t + 1) * P, :], in_=ot[:])

    nc.compile()
    _cached_nc = nc
    return nc


def _prepare_inputs(inputs, w0, w, v):
    dense = np.ascontiguousarray(inputs[:, :N_DENSE].astype(np.float32))
    idx = inputs[:, N_DENSE:].astype(np.int32)
    flat_idx = (N_DENSE + np.arange(N_SPARSE, dtype=np.int32) * ONEHOT)[None, :] + idx

    table = np.zeros((FEAT, E), np.float16)
    table[:, :D] = v.reshape(FEAT, D).astype(np.float16)
    table[:, D] = np.asarray(w, np.float32).reshape(FEAT).astype(np.float16)
    w0_row = np.zeros((1, DW), np.float32)
    w0_row[0, D] = np.asarray(w0, np.float32).reshape(-1)[0]
    vdx_top = np.concatenate(
        [v.reshape(FEAT, D)[:N_DENSE], np.asarray(w, np.float32).reshape(FEAT, 1)[:N_DENSE]],
        axis=1,
    ).astype(np.float32)
    vdx = np.ascontiguousarray(np.concatenate([vdx_top, w0_row], axis=0))

    in_maps = []
    for c in range(NCORES):
        sl = slice(c * BC, (c + 1) * BC)
        dnt = np.concatenate(
            [dense[sl].T, np.ones((1, BC), np.float32)], axis=0
        )  # [14, 512]
        # per tile t the gather consumes indices i = c*128 + p, laid out
        # int16 at [i % 16, i // 16] in the first 16 partitions, replicated
        # 8x down the partitions (one copy per Q7 core)
        fi = flat_idx[sl].astype(np.int16)  # [512, 26]
        blocks = []
        for t in range(NT):
            for h in range(2):
                # half h covers fields 13h..13h+12; order i = c_local*128 + p
                lin = fi[t * P:(t + 1) * P, 13 * h:13 * (h + 1)].T.reshape(NI // 2)
                blk = lin.reshape(NI // 32, 16).T  # [16, HNI/16]
                blocks.append(np.tile(blk, (8, 1)))  # [128, HNI/16]
        idx_buf = np.ascontiguousarray(np.concatenate(blocks, axis=1))
        in_maps.append(
            {
                "table": table,
                "idx": idx_buf,
                "dnt": np.ascontiguousarray(dnt),
                "vdx": vdx,
            }
        )
    return in_maps


def kernel(**inputs):
    from concourse import bass_utils

    nc = _build_program()
    in_maps = _prepare_inputs(
        np.asarray(inputs["inputs"]),
        np.asarray(inputs["w0"]),
        np.asarray(inputs["w"]),
        np.asarray(inputs["v"]),
    )
    res = bass_utils.run_bass_kernel_spmd(nc, in_maps, core_ids=list(range(NCORES)))
    outs = [np.asarray(res.results[c]["out"]) for c in range(NCORES)]
    return np.concatenate(outs, axis=0).astype(np.float32)


# revision 18
# speedup vs baseline: 2.2078x; 1.0417x over previous
"""FFM layer kernel for Trainium2, data-parallel over batch on 8 NeuronCores.

The reference computes, for each sample b:
    x = [dense(13) | onehot(26 fields x 1000)]            # [B, 26013]
    linear = w0 + x @ w                                   # [B, 1]
    field_f = einsum('bf,fik->bik', x, v)                 # [B, 39, 8]
    inter = 0.5*((sum_i field_f)^2.sum(k) - (field_f^2).sum(i,k))
    out = linear + inter

Because x is one-hot in the sparse block, x @ [v|w] is a 26-row gather from
a [26013, 320] table (cols 0..311 = flattened v row, col 312 = w, 313.. pad)
plus a tiny dense [14]x[14,313] matmul (row 13 = ones row carrying w0 into
col 312).  Each core handles 512 samples as 4 tiles of 128; each tile's 26
rows/sample are fetched by one gpsimd dma_gather (3328 rows of 1280 B).
"""

import numpy as np

N_DENSE = 13
N_SPARSE = 26
ONEHOT = 1000
FIELD = 39
K = 8
FEAT = N_DENSE + N_SPARSE * ONEHOT  # 26013
B = 4096
NCORES = 8
BC = B // NCORES  # 512 samples per core
P = 128
NT = BC // P  # 4 tiles per core
D = FIELD * K  # 312
DW = D + 1  # 313 (col 312 carries the linear weight)
E = 384  # gathered fp16 row width, padded so the 768 B row is a multiple of 256
NI = N_SPARSE * P  # 3328 indices per tile gather
SQRT_HALF = 0.7071067811865476

_cached_nc = None


def _build_program():
    global _cached_nc
    if _cached_nc is not None:
        return _cached_nc

    import concourse.bacc as bacc
    import concourse.mybir as mybir
    from concourse.tile import TileContext
    from concourse import library_config

    nc = bacc.Bacc(
        "TRN2",
        debug=False,
        enable_asserts=False,
        target_bir_lowering=False,
        num_devices=NCORES,
        num_swdge_queues=4,
        dynamic_dma_scratch_size=32768,
    )
    f32 = mybir.dt.float32
    f16 = mybir.dt.float16
    i16 = mybir.dt.int16
    table = nc.dram_tensor("table", [FEAT, E], f16, kind="ExternalInput").ap()
    idx = nc.dram_tensor("idx", [P, NT * NI // 16], i16, kind="ExternalInput").ap()
    dnt = nc.dram_tensor("dnt", [N_DENSE + 1, BC], f32, kind="ExternalInput").ap()
    vdx = nc.dram_tensor("vdx", [N_DENSE + 1, DW], f32, kind="ExternalInput").ap()
    out = nc.dram_tensor("out", [BC, 1], f32, kind="ExternalOutput").ap()

    with TileContext(nc) as tc:
        with tc.tile_pool(name="const", bufs=1) as cpool, \
             tc.tile_pool(name="gath", bufs=NT) as gpool, \
             tc.tile_pool(name="work", bufs=2) as wpool, \
             tc.tile_pool(name="psum", bufs=2, space="PSUM") as ppool:
            nc.gpsimd.load_library(library_config.mlp)
            idx_sb = cpool.tile([P, NT * NI // 16], i16)
            nc.sync.dma_start(out=idx_sb[:], in_=idx[:])
            dnt_sb = cpool.tile([N_DENSE + 1, BC], f32)
            nc.sync.dma_start(out=dnt_sb[:], in_=dnt[:])
            vdx_sb = cpool.tile([N_DENSE + 1, DW], f32)
            nc.sync.dma_start(out=vdx_sb[:], in_=vdx[:])

            qn = 0
            for t in range(NT):
                # gh[h] covers 13 fields as two sub-gathers (7+6 fields) on
                # rotating SWDGE queues for finer stream interleaving
                HC = (NI // 2) // 16  # idx columns per 13-field half
                gh = []
                for h in range(2):
                    g = gpool.tile([P, 13 * E], f16, tag=f"g{h}")
                    g3 = g[:].rearrange("p (c e) -> p c e", e=E)
                    col = (2 * t + h) * HC
                    for (c0, nf) in ((0, 5), (5, 4), (9, 4)):
                        ni = nf * P
                        nc.gpsimd.dma_gather(
                            g3[:, c0:c0 + nf, :],
                            table[:],
                            idx_sb[:, col + c0 * 8:col + (c0 + nf) * 8],
                            ni,
                            ni,
                            E,
                            single_packet=False,
                            queue_num=qn % 4,
                        )
                        qn += 1
                    gh.append(g)
                # dense + w0 contribution: [128, 313]
                ps = ppool.tile([P, DW], f32, tag="ps")
                nc.tensor.matmul(
                    out=ps[:],
                    lhsT=dnt_sb[:, t * P:(t + 1) * P],
                    rhs=vdx_sb[:],
                    start=True,
                    stop=True,
                )
                # sum the 26 gathered rows with a contiguous pairwise tree
                # (fp32 tensor_tensor runs 1 elem/cycle; contiguous > strided)
                add = lambda o, a, b: nc.vector.tensor_tensor(
                    out=o, in0=a, in1=b, op=mybir.AluOpType.add
                )
                # fp16 halves collapse 13 blocks -> fp32 partials -> 1 block;
                # all adds use 313-wide views so the 71 pad columns per block
                # are never touched by the vector engine
                W = DW  # 313 payload columns per 384-wide block
                a6 = []
                for h in range(2):
                    g3 = gh[h][:].rearrange("p (c e) -> p c e", e=E)
                    a = wpool.tile([P, 6 * E], f32, tag=f"a6_{h}")
                    a3 = a[:].rearrange("p (c e) -> p c e", e=E)
                    add(a3[:, 0:6, 0:W], g3[:, 0:6, 0:W], g3[:, 6:12, 0:W])
                    add(a3[:, 0:3, 0:W], a3[:, 0:3, 0:W], a3[:, 3:6, 0:W])
                    a6.append(a)
                a03 = a6[0][:].rearrange("p (c e) -> p c e", e=E)
                a13 = a6[1][:].rearrange("p (c e) -> p c e", e=E)
                g03 = gh[0][:].rearrange("p (c e) -> p c e", e=E)
                g13 = gh[1][:].rearrange("p (c e) -> p c e", e=E)
                add(a03[:, 0:3, 0:W], a03[:, 0:3, 0:W], a13[:, 0:3, 0:W])
                add(a03[:, 0, 0:W], a03[:, 0, 0:W], a03[:, 1, 0:W])
                add(a03[:, 0, 0:W], a03[:, 0, 0:W], a03[:, 2, 0:W])
                add(a03[:, 0, 0:W], a03[:, 0, 0:W], g03[:, 12, 0:W])
                add(a03[:, 0, 0:W], a03[:, 0, 0:W], g13[:, 12, 0:W])
                tot = wpool.tile([P, DW], f32, tag="tot")
                add(tot[:], a6[0][:, :DW], ps[:])
                # s_k = sum_i field_f[i, k]: view [P, 8, 39], reduce innermost
                s8 = wpool.tile([P, K], f32, tag="s8")
                tv = tot[:, :D].rearrange("p (i k) -> p k i", k=K)
                nc.vector.reduce_sum(out=s8[:], in_=tv, axis=mybir.AxisListType.X)
                # 0.5 * sum of squares, fused on the scalar engine
                sq = wpool.tile([P, D], f32, tag="sq")
                h_sumsq = wpool.tile([P, 1], f32, tag="h_sumsq")
                nc.scalar.activation(
                    out=sq[:], in_=tot[:, :D],
                    func=mybir.ActivationFunctionType.Square,
                    scale=SQRT_HALF, accum_out=h_sumsq[:],
                )
                sq8 = wpool.tile([P, K], f32, tag="sq8")
                h_ssq = wpool.tile([P, 1], f32, tag="h_ssq")
                nc.scalar.activation(
                    out=sq8[:], in_=s8[:],
                    func=mybir.ActivationFunctionType.Square,
                    scale=SQRT_HALF, accum_out=h_ssq[:],
                )
                ot = wpool.tile([P, 1], f32, tag="ot")
                nc.vector.tensor_tensor(
                    out=ot[:], in0=h_ssq[:], in1=h_sumsq[:],
                    op=mybir.AluOpType.subtract,
                )
                add(ot[:], ot[:], tot[:, D:DW])
                nc.sync.dma_start(out=out[t * P:(t + 1) * P, :], in_=ot[:])

    nc.compile()
    _cached_nc = nc
    return nc


def _prepare_inputs(inputs, w0, w, v):
    dense = np.ascontiguousarray(inputs[:, :N_DENSE].astype(np.float32))
    idx = inputs[:, N_DENSE:].astype(np.int32)
    flat_idx = (N_DENSE + np.arange(N_SPARSE, dtype=np.int32) * ONEHOT)[None, :] + idx

    table = np.zeros((FEAT, E), np.float16)
    table[:, :D] = v.reshape(FEAT, D).astype(np.float16)
    table[:, D] = np.asarray(w, np.float32).reshape(FEAT).astype(np.float16)
    w0_row = np.zeros((1, DW), np.float32)
    w0_row[0, D] = np.asarray(w0, np.float32).reshape(-1)[0]
    vdx_top = np.concatenate(
        [v.reshape(FEAT, D)[:N_DENSE], np.asarray(w, np.float32).reshape(FEAT, 1)[:N_DENSE]],
        axis=1,
    ).astype(np.float32)
    vdx = np.ascontiguousarray(np.concatenate([vdx_top, w0_row], axis=0))

    in_maps = []
    for c in range(NCORES):
        sl = slice(c * BC, (c + 1) * BC)
        dnt = np.concatenate(
            [dense[sl].T, np.ones((1, BC), np.float32)], axis=0
        )  # [14, 512]
        # per tile t the gather consumes indices i = c*128 + p, laid out
        # int16 at [i % 16, i // 16] in the first 16 partitions, replicated
        # 8x down the partitions (one copy per Q7 core)
        fi = flat_idx[sl].astype(np.int16)  # [512, 26]
        blocks = []
        for t in range(NT):
            for h in range(2):
                # half h covers fields 13h..13h+12; order i = c_local*128 + p
                lin = fi[t * P:(t + 1) * P, 13 * h:13 * (h + 1)].T.reshape(NI // 2)
                blk = lin.reshape(NI // 32, 16).T  # [16, HNI/16]
                blocks.append(np.tile(blk, (8, 1)))  # [128, HNI/16]
        idx_buf = np.ascontiguousarray(np.concatenate(blocks, axis=1))
        in_maps.append(
            {
                "table": table,
                "idx": idx_buf,
                "dnt": np.ascontiguousarray(dnt),
                "vdx": vdx,
            }
        )
    return in_maps


def kernel(**inputs):
    from concourse import bass_utils

    nc = _build_program()
    in_maps = _prepare_inputs(
        np.asarray(inputs["inputs"]),
        np.asarray(inputs["w0"]),
        np.asarray(inputs["w"]),
        np.asarray(inputs["v"]),
    )
    res = bass_utils.run_bass_kernel_spmd(nc, in_maps, core_ids=list(range(NCORES)))
    outs = [np.asarray(res.results[c]["out"]) for c in range(NCORES)]
    return np.concatenate(outs, axis=0).astype(np.float32)
